# revision 6
# baseline (speedup 1.0000x reference)
"""Trainium2 Bass kernel for nn_Net_29102698398058 (gnn_message_passing).

Strategy (8 NeuronCores, SPMD single program, per-core data):
  - data_vecs is sharded row-wise (vocab/8 per core). Each core gathers the
    rows of its local nodes via indirect DMA, computes a partial leaf
    embedding, and an AllReduce combines partials into the full leaf [128,64].
  - Core j runs the tree DP for graph j. The tree structure (which differs
    per core) is encoded purely in per-core DATA (placement / adjacency /
    harvest matrices and masks) so the instruction stream is identical on all
    cores: per level, messages are computed with per-edge-type matmuls
    (weights compile-time indexed, since `edges` is shared by all graphs) and
    routed to parents with an adjacency matmul against per-core 0/1 matrices.
  - The 16 edge candidates only differ at node `pos`, so the DP is factored:
    a single shared DP over non-path nodes, then a short per-candidate
    correction along the path pos -> root, batched over all 16 candidates.
"""

import os
import numpy as np

T, V, VD = 2, 100000, 300
N, G, E, DIM = 64, 8, 16, 128
NC = 8
F32 = np.float32

LAST_EXEC_NS = None


# --------------------------------------------------------------------------
# host-side structure / schedule
# --------------------------------------------------------------------------

def _build_host(data, types, graphs, edges, pos):
    Vs = V // NC
    perm = np.argsort(types, kind="stable")          # row p <-> node perm[p]
    perm_inv = np.argsort(perm)
    n0 = int((types == 0).sum())

    gs = []
    for j in range(G):
        par = np.arange(N) + graphs[j]
        depth = np.zeros(N, dtype=np.int64)
        for i in range(1, N):
            depth[i] = depth[par[i]] + 1
        hc = np.zeros(N, dtype=bool)
        hc[par[1:]] = True
        path = [pos]
        while path[-1] != 0:
            path.append(int(par[path[-1]]))
        gs.append(dict(par=par, depth=depth, hc=hc, path=path, D=int(depth.max())))

    R = max(g["D"] for g in gs)
    PI = max(len(g["path"]) - 2 for g in gs)         # padded intermediate steps
    HC = PI + 2                                      # harvest cols: PI inter, root, pos

    # universal (round, edge) slot sizes; path nodes excluded
    members = {}
    for j, g in enumerate(gs):
        pathset = set(g["path"])
        for i in range(1, N):
            if i in pathset:
                continue
            r = R - int(g["depth"][i])
            members.setdefault((j, r, int(edges[i])), []).append(i)
    groups = []          # (r, e, col_start, count)
    round_cols = {}
    col = 0
    for r in range(R):
        rstart = col
        for e in range(E):
            cnt = max((len(members.get((j, r, e), [])) for j in range(G)), default=0)
            if cnt:
                groups.append((r, e, col, cnt))
                col += cnt
        round_cols[r] = (rstart, col - rstart)
    M = col
    nonempty = [r for r in range(R) if round_cols[r][1] > 0]

    # adjacency column packing (for rounds whose NEXT round is nonempty)
    adj_off = {}
    acol = 0
    for r in range(R - 1):
        rs = round_cols[r][1]
        ns = round_cols[r + 1][1]
        if rs > 0 and ns > 0:
            adj_off[r] = acol
            acol += ns
    AC = max(acol, 1)

    correct = int(edges[pos])
    others = [e for e in range(E) if e != correct]
    leftover = others[-1]

    cores = []
    for j in range(G):
        g = gs[j]
        par, depth, hc, path = g["par"], g["depth"], g["hc"], g["path"]
        L = len(path) - 1                    # number of messages pos->root
        pathset = set(path)
        slot_of = {}
        for (r, e, c0, cnt) in groups:
            for k, node in enumerate(members.get((j, r, e), [])):
                slot_of[node] = c0 + k
        # leaf placement [64(perm rows), M + HC]
        P = np.zeros((N, M + HC), F32)
        for node, s in slot_of.items():
            P[node, s] = 1.0
        for i in range(1, L):                # intermediates p_1..p_{L-1}
            P[path[i], M + i - 1] = 1.0
        P[0, M + PI] = 1.0                   # root
        P[pos, M + PI + 1] = 1.0            # pos
        P = P[perm]
        # hc mask over slots, broadcast to 128 partitions
        hcm = np.zeros(max(M, 1), F32)
        for node, s in slot_of.items():
            if hc[node]:
                hcm[s] = 1.0
        hcm128 = np.broadcast_to(hcm, (128, max(M, 1))).astype(np.uint8)
        # adjacency + harvest
        adj = np.zeros((128, AC), F32)
        hmat = np.zeros((128, max(R, 1) * HC), F32)
        for node, s in slot_of.items():
            r = R - int(depth[node])
            r0 = round_cols[r][0]
            p = int(par[node])
            if p == 0:
                hmat[s - r0, r * HC + PI] = 1.0
            elif p == pos:
                hmat[s - r0, r * HC + PI + 1] = 1.0
            elif p in pathset:
                i = path.index(p)            # 1..L-1
                hmat[s - r0, r * HC + i - 1] = 1.0
            else:
                n0_ = round_cols[r + 1][0]
                adj[s - r0, adj_off[r] + slot_of[p] - n0_] = 1.0
        # path data
        pewidx = np.zeros((128, max(PI, 1)), np.int32)
        ebsel = np.zeros((E + 1, max(PI, 1)), F32)
        pmask = np.zeros((128, max(PI, 1) * E), np.uint8)
        for i in range(1, PI + 1):
            if i <= L - 1:
                e_i = int(edges[path[i]])
                pewidx[:, i - 1] = e_i * 128 + np.arange(128)
                ebsel[e_i, i - 1] = 1.0
                pmask[:, (i - 1) * E:i * E] = 1
            else:
                pewidx[:, i - 1] = E * 128 + np.arange(128)   # identity block
        # candidate map
        first = correct if j == 0 else leftover
        cand = [first] + others
        C = np.zeros((E, E), F32)
        for k, e in enumerate(cand):
            C[e, k] = 1.0
        # pos relu mask (hc[pos] is almost always False, but be general)
        hpos = np.full((128, 1), 1 if hc[pos] else 0, np.uint8)
        # gather indices + local mask (rows in perm order)
        c = j  # core index == graph index
        gidx = np.zeros((N, 1), np.int32)
        lmask = np.zeros((N, 1), F32)
        for prow in range(N):
            node = perm[prow]
            d = int(data[node])
            if c * Vs <= d < (c + 1) * Vs:
                gidx[prow, 0] = int(types[node]) * Vs + (d - c * Vs)
                lmask[prow, 0] = 1.0
        cores.append(dict(P=P, hcm=hcm128, adj=adj, hmat=hmat, pewidx=pewidx,
                          ebsel=ebsel, pmask=pmask, C=C, hpos=hpos,
                          gidx=gidx, lmask=lmask))

    typesel = np.zeros((2, N), F32)
    for prow in range(N):
        typesel[int(types[perm[prow]]), prow] = 1.0

    return dict(R=R, PI=PI, HC=HC, M=M, AC=AC, groups=groups,
                round_cols=round_cols, nonempty=nonempty, adj_off=adj_off,
                perm=perm, n0=n0, colpos=int(perm_inv[pos]),
                typesel=typesel, cores=cores)


# --------------------------------------------------------------------------
# device program
# --------------------------------------------------------------------------

def _build_program(H):
    import concourse.bass as bass
    import concourse.tile as tile
    from concourse import bacc, mybir
    from concourse.masks import make_identity

    f32 = mybir.dt.float32
    i32 = mybir.dt.int32
    AF = mybir.ActivationFunctionType
    R, PI, HC, M, AC = H["R"], H["PI"], H["HC"], H["M"], H["AC"]
    n0, colpos = H["n0"], H["colpos"]
    Vs = V // NC

    nc = bacc.Bacc("TRN2", target_bir_lowering=False, debug=False,
                   enable_asserts=True, num_devices=NC)

    def din(name, shape, dt=f32):
        return nc.dram_tensor(name, shape, dt, kind="ExternalInput").ap()

    dv = din("dv", [2 * Vs, VD])
    dw = din("dw", [2, VD, DIM])
    db = din("db", [2, DIM])
    ew_ext = din("ew_ext", [(E + 1) * 128, 128])
    ew_t = din("ew_t", [128, E * 128])
    eb_ext = din("eb_ext", [E + 1, 128])
    ebt = din("ebt", [128, E])
    sew = din("sew", [DIM, 1])
    sdw = din("sdw", [DIM, 1])
    sebcol = din("sebcol", [DIM, 1])
    sdbcol = din("sdbcol", [DIM, 1])
    gidx = din("gidx", [N, 1], i32)
    lmask = din("lmask", [N, 1])
    pmat = din("pmat", [N, M + HC])
    tsel = din("tsel", [2, N])
    hcm = din("hcm", [128, max(M, 1)], mybir.dt.uint8)
    adj = din("adj", [128, AC])
    hmat = din("hmat", [128, max(R, 1) * HC])
    pewidx = din("pewidx", [128, max(PI, 1)], i32)
    ebsel = din("ebsel", [E + 1, max(PI, 1)])
    pmask = din("pmask", [128, max(PI, 1) * E], mybir.dt.uint8)
    cmat = din("cmat", [E, E])
    hpos = din("hpos", [128, 1], mybir.dt.uint8)
    out = nc.dram_tensor("out", [1, E], f32, kind="ExternalOutput").ap()

    with tile.TileContext(nc) as tc:
        with (
            tc.tile_pool(name="const", bufs=1) as pc,
            tc.tile_pool(name="work", bufs=2) as pw,
            tc.tile_pool(name="pmisc", bufs=1, space="PSUM") as pmisc,
            tc.tile_pool(name="pacc", bufs=2, space="PSUM") as pacc,
            tc.tile_pool(name="pmsg", bufs=2, space="PSUM") as pmsg,
            tc.tile_pool(name="prow", bufs=1, space="PSUM") as prow,
            tc.tile_pool(name="ppath", bufs=1, space="PSUM") as ppath,
            tc.tile_pool(name="dram", bufs=1, space="DRAM") as pdram,
        ):
            # ---- constants / weights to SBUF ----
            ident = pc.tile([128, 128], f32)
            make_identity(nc, ident[:])
            ew_sb = pc.tile([128, E * 128], f32)
            nc.sync.dma_start(ew_sb[:], ew_t[:])
            eb_sb = pc.tile([E + 1, 128], f32)
            nc.sync.dma_start(eb_sb[:], eb_ext[:])
            ebt_sb = pc.tile([128, E], f32)
            nc.sync.dma_start(ebt_sb[:], ebt[:])
            dw_sb = []
            chunks = [(0, 128), (128, 128), (256, VD - 256)]
            for t in range(2):
                row = []
                for (c0, ck) in chunks:
                    wt = pc.tile([ck, DIM], f32, tag=f"dw{t}{c0}")
                    nc.sync.dma_start(wt[:], dw[t, c0:c0 + ck, :])
                    row.append(wt)
                dw_sb.append(row)
            db_sb = pc.tile([2, DIM], f32)
            nc.sync.dma_start(db_sb[:], db[:])
            sew_sb = pc.tile([DIM, 1], f32)
            nc.sync.dma_start(sew_sb[:], sew[:])
            sdw_sb = pc.tile([DIM, 1], f32)
            nc.sync.dma_start(sdw_sb[:], sdw[:])
            seb_sb = pc.tile([DIM, 1], f32)
            nc.sync.dma_start(seb_sb[:], sebcol[:])
            sdb_sb = pc.tile([DIM, 1], f32)
            nc.sync.dma_start(sdb_sb[:], sdbcol[:])
            gidx_sb = pc.tile([N, 1], i32)
            nc.sync.dma_start(gidx_sb[:], gidx[:])
            lmask_sb = pc.tile([N, 1], f32)
            nc.sync.dma_start(lmask_sb[:], lmask[:])
            pmat_sb = pc.tile([N, M + HC], f32)
            nc.sync.dma_start(pmat_sb[:], pmat[:])
            tsel_sb = pc.tile([2, N], f32)
            nc.sync.dma_start(tsel_sb[:], tsel[:])
            hcm_sb = pc.tile([128, max(M, 1)], mybir.dt.uint8)
            nc.sync.dma_start(hcm_sb[:], hcm[:])
            adj_sb = pc.tile([128, AC], f32)
            nc.sync.dma_start(adj_sb[:], adj[:])
            hmat_sb = pc.tile([128, max(R, 1) * HC], f32)
            nc.sync.dma_start(hmat_sb[:], hmat[:])
            ebsel_sb = pc.tile([E + 1, max(PI, 1)], f32)
            nc.sync.dma_start(ebsel_sb[:], ebsel[:])
            pmask_sb = pc.tile([128, max(PI, 1) * E], mybir.dt.uint8)
            nc.sync.dma_start(pmask_sb[:], pmask[:])
            cmat_sb = pc.tile([E, E], f32)
            nc.sync.dma_start(cmat_sb[:], cmat[:])
            hpos_sb = pc.tile([128, 1], mybir.dt.uint8)
            nc.sync.dma_start(hpos_sb[:], hpos[:])
            pewidx_sb = pc.tile([128, max(PI, 1)], i32)
            nc.sync.dma_start(pewidx_sb[:], pewidx[:])
            pew_sb = []
            for i in range(PI):
                pt = pc.tile([128, 128], f32, tag=f"pew{i}")
                nc.gpsimd.indirect_dma_start(
                    out=pt[:], out_offset=None, in_=ew_ext[:],
                    in_offset=bass.IndirectOffsetOnAxis(ap=pewidx_sb[:, i:i + 1], axis=0),
                )
                pew_sb.append(pt)

            # ---- leaf phase ----
            vecs = pw.tile([N, VD], f32)
            nc.gpsimd.indirect_dma_start(
                out=vecs[:], out_offset=None, in_=dv[:],
                in_offset=bass.IndirectOffsetOnAxis(ap=gidx_sb[:, 0:1], axis=0),
            )
            vecs_m = pw.tile([N, VD], f32)
            nc.vector.tensor_scalar_mul(vecs_m[:], vecs[:], lmask_sb[:, 0:1])
            vT_sb = []
            for (c0, ck) in chunks:
                tp = pmisc.tile([ck, N], f32, tag="mp")
                nc.tensor.transpose(tp[:], vecs_m[:, c0:c0 + ck], ident[0:N, 0:N])
                ts = pw.tile([ck, N], f32, tag=f"vT{c0}")
                nc.vector.tensor_copy(ts[:], tp[:])
                vT_sb.append(ts)
            leafpart = pmisc.tile([DIM, N], f32, tag="mp")
            for t, (cl, cr) in enumerate([(0, n0), (n0, N)]):
                if cr - cl == 0:
                    continue
                for ci, (c0, ck) in enumerate(chunks):
                    nc.tensor.matmul(leafpart[:, cl:cr], dw_sb[t][ci][:],
                                     vT_sb[ci][:, cl:cr],
                                     start=(ci == 0), stop=(ci == len(chunks) - 1))
            leafpart_sb = pw.tile([DIM, N], f32)
            nc.vector.tensor_copy(leafpart_sb[:], leafpart[:])
            lf_in = pdram.tile([DIM, N], f32)
            lf_out = pdram.tile([DIM, N], f32)
            nc.gpsimd.dma_start(lf_in[:], leafpart_sb[:])
            nc.gpsimd.collective_compute(
                "AllReduce", bass.mybir.AluOpType.add,
                replica_groups=[list(range(NC))],
                ins=[lf_in.opt()], outs=[lf_out.opt()],
            )
            leaf0_sb = pw.tile([DIM, N], f32)
            nc.gpsimd.dma_start(leaf0_sb[:], lf_out[:])
            biascols = pmisc.tile([DIM, N], f32, tag="mp")
            nc.tensor.matmul(biascols[:], db_sb[:], tsel_sb[:], start=True, stop=True)
            leaf_sb = pc.tile([DIM, N], f32)
            nc.vector.tensor_add(leaf_sb[:], leaf0_sb[:], biascols[:])
            leafrow_p = pmisc.tile([N, DIM], f32, tag="mp")
            nc.tensor.transpose(leafrow_p[:], leaf_sb[:], ident[:])
            leafrow_sb = pw.tile([N, DIM], f32)
            nc.vector.tensor_copy(leafrow_sb[:], leafrow_p[:])

            # ---- slot placement ----
            slots_p = pmisc.tile([DIM, M + HC], f32, tag="mp")
            nc.tensor.matmul(slots_p[:], leafrow_sb[:], pmat_sb[:], start=True, stop=True)
            lslots = pc.tile([DIM, M + HC], f32)
            nc.vector.tensor_copy(lslots[:], slots_p[:])

            # ---- shared DP rounds ----
            pathacc = ppath.tile([DIM, HC], f32, name="pathacc", tag="pathacc") if M > 0 else None
            ne = H["nonempty"]
            acc_prev = None   # (psum_tile, size) from previous round
            for r in range(R):
                r0, rs = H["round_cols"][r]
                if rs == 0:
                    acc_prev = None
                    continue
                e_sb = pw.tile([DIM, rs], f32, tag="e_r")
                if acc_prev is None:
                    nc.vector.tensor_copy(e_sb[:], lslots[:, r0:r0 + rs])
                else:
                    nc.vector.tensor_add(e_sb[:], lslots[:, r0:r0 + rs], acc_prev[:])
                relu_sb = pw.tile([DIM, rs], f32, tag="relu_r")
                nc.scalar.activation(relu_sb[:], e_sb[:], AF.Relu)
                nc.vector.copy_predicated(e_sb[:], hcm_sb[:, r0:r0 + rs], relu_sb[:])
                msg_p = pmsg.tile([DIM, rs], f32, tag="msg")
                for (rr, e, c0, cnt) in H["groups"]:
                    if rr != r:
                        continue
                    lc = c0 - r0
                    nc.tensor.matmul(msg_p[:, lc:lc + cnt],
                                     ew_sb[:, e * 128:(e + 1) * 128],
                                     e_sb[:, lc:lc + cnt], start=True, stop=True)
                msgb_sb = pw.tile([DIM, rs], f32, tag="msgb")
                for (rr, e, c0, cnt) in H["groups"]:
                    if rr != r:
                        continue
                    lc = c0 - r0
                    nc.scalar.activation(msgb_sb[:, lc:lc + cnt], msg_p[:, lc:lc + cnt],
                                         AF.Identity, bias=ebt_sb[:, e:e + 1])
                row_p = prow.tile([rs, DIM], f32, tag="mrow")
                nc.tensor.transpose(row_p[:], msgb_sb[:], ident[:])
                row_sb = pw.tile([rs, DIM], f32, tag="mrowsb")
                nc.vector.tensor_copy(row_sb[:], row_p[:])
                if r in H["adj_off"]:
                    ns = H["round_cols"][r + 1][1]
                    ao = H["adj_off"][r]
                    acc_new = pacc.tile([DIM, ns], f32, tag="acc")
                    nc.tensor.matmul(acc_new[:], row_sb[:], adj_sb[0:rs, ao:ao + ns],
                                     start=True, stop=True)
                    acc_prev = acc_new
                else:
                    acc_prev = None
                nc.tensor.matmul(pathacc[:], row_sb[:], hmat_sb[0:rs, r * HC:(r + 1) * HC],
                                 start=(r == ne[0]), stop=(r == ne[-1]),
                                 skip_group_check=True)

            # padd = pathleafs + pathacc  [128, HC]
            padd = pc.tile([DIM, HC], f32)
            if pathacc is not None:
                nc.vector.tensor_add(padd[:], lslots[:, M:M + HC], pathacc[:])
            else:
                nc.vector.tensor_copy(padd[:], lslots[:, M:M + HC])

            # ---- candidate init at pos ----
            epos = pw.tile([DIM, 1], f32)
            nc.vector.tensor_copy(epos[:], padd[:, PI + 1:PI + 2])
            eposr = pw.tile([DIM, 1], f32)
            nc.scalar.activation(eposr[:], epos[:], AF.Relu)
            nc.vector.copy_predicated(epos[:], hpos_sb[:], eposr[:])
            m63_p = pmisc.tile([DIM, E], f32, tag="mp")
            for e in range(E):
                nc.tensor.matmul(m63_p[:, e:e + 1], ew_sb[:, e * 128:(e + 1) * 128],
                                 epos[:], start=True, stop=True)
            m63_sb = pw.tile([DIM, E], f32)
            nc.vector.tensor_copy(m63_sb[:], m63_p[:])
            m63row_p = pmisc.tile([E, DIM], f32, tag="mp")
            nc.tensor.transpose(m63row_p[:], m63_sb[:], ident[:])
            m63row_sb = pw.tile([E, DIM], f32)
            nc.vector.tensor_copy(m63row_sb[:], m63row_p[:])
            m0a = pmisc.tile([DIM, E], f32, tag="mp")
            nc.tensor.matmul(m0a[:], m63row_sb[:], cmat_sb[:], start=True, stop=False,
                             skip_group_check=True)
            nc.tensor.matmul(m0a[:], eb_sb[0:E, :], cmat_sb[:], start=False, stop=True,
                             skip_group_check=True)
            m_sb = pw.tile([DIM, E], f32, tag="m_path")
            nc.vector.tensor_copy(m_sb[:], m0a[:])

            # path bias columns
            ebpath_p = pmsg.tile([DIM, max(PI, 1)], f32, tag="msg")
            nc.tensor.matmul(ebpath_p[:], eb_sb[:], ebsel_sb[:], start=True, stop=True)
            ebpath_sb = pw.tile([DIM, max(PI, 1)], f32)
            nc.vector.tensor_copy(ebpath_sb[:], ebpath_p[:])

            # ---- path steps ----
            for i in range(1, PI + 1):
                e_i = pw.tile([DIM, E], f32, tag="e_path")
                nc.vector.tensor_add(e_i[:], m_sb[:],
                                     padd[:, i - 1:i].to_broadcast([DIM, E]))
                r_i = pw.tile([DIM, E], f32, tag="r_path")
                nc.scalar.activation(r_i[:], e_i[:], AF.Relu)
                nc.vector.copy_predicated(e_i[:], pmask_sb[:, (i - 1) * E:i * E], r_i[:])
                mi_p = pmisc.tile([DIM, E], f32, tag="mp")
                nc.tensor.matmul(mi_p[:], pew_sb[i - 1][:], e_i[:], start=True, stop=True)
                m_sb = pw.tile([DIM, E], f32, tag="m_path")
                nc.scalar.activation(m_sb[:], mi_p[:], AF.Identity,
                                     bias=ebpath_sb[:, i - 1:i])

            # ---- root + scores ----
            eroot = pw.tile([DIM, E], f32)
            nc.vector.tensor_add(eroot[:], m_sb[:],
                                 padd[:, PI:PI + 1].to_broadcast([DIM, E]))
            roots = pw.tile([DIM, E], f32)
            nc.scalar.activation(roots[:], eroot[:], AF.Relu)
            rootspb = pw.tile([DIM, E], f32)
            nc.vector.tensor_add(rootspb[:], roots[:],
                                 seb_sb[:, 0:1].to_broadcast([DIM, E]))
            s_p = pmisc.tile([1, E], f32, tag="mp")
            nc.tensor.matmul(s_p[:], sew_sb[:], rootspb[:], start=True, stop=True)
            demb = pw.tile([DIM, 1], f32)
            nc.vector.tensor_add(demb[:], leaf_sb[:, colpos:colpos + 1], sdb_sb[:])
            c_p = pmsg.tile([1, 1], f32, tag="msg")
            nc.tensor.matmul(c_p[:], sdw_sb[:], demb[:], start=True, stop=True)
            c_sb = pw.tile([1, 1], f32)
            nc.vector.tensor_copy(c_sb[:], c_p[:])
            s_sb = pw.tile([1, E], f32)
            nc.vector.tensor_add(s_sb[:], s_p[:], c_sb[:, 0:1].to_broadcast([1, E]))
            nc.sync.dma_start(out[:], s_sb[:])

    nc.compile()
    return nc


# --------------------------------------------------------------------------
# entry point
# --------------------------------------------------------------------------

def kernel(**inputs):
    global LAST_EXEC_NS
    data = np.asarray(inputs["data"]).astype(np.int64)
    types = np.asarray(inputs["types"]).astype(np.int64)
    graphs = np.asarray(inputs["graphs"]).astype(np.int64)
    edges = np.asarray(inputs["edges"]).astype(np.int64)
    pos = int(np.asarray(inputs["pos"]))
    dv = np.ascontiguousarray(np.asarray(inputs["data_vecs"], dtype=F32))
    dw = np.ascontiguousarray(np.asarray(inputs["data_weights"], dtype=F32))
    db = np.ascontiguousarray(np.asarray(inputs["data_biases"], dtype=F32))
    ew = np.ascontiguousarray(np.asarray(inputs["edge_weights"], dtype=F32))
    eb = np.ascontiguousarray(np.asarray(inputs["edge_biases"], dtype=F32))
    sew = np.ascontiguousarray(np.asarray(inputs["score_embedding_weights"], dtype=F32))
    seb = np.ascontiguousarray(np.asarray(inputs["score_embedding_biases"], dtype=F32))
    sdw = np.ascontiguousarray(np.asarray(inputs["score_data_weights"], dtype=F32))
    sdb = np.ascontiguousarray(np.asarray(inputs["score_data_biases"], dtype=F32))

    H = _build_host(data, types, graphs, edges, pos)
    nc = _build_program(H)

    Vs = V // NC
    ew_ext = np.concatenate([ew.reshape(E * 128, 128),
                             np.eye(128, dtype=F32)], axis=0)
    ew_t = np.ascontiguousarray(ew.transpose(1, 0, 2).reshape(128, E * 128))
    eb_ext = np.concatenate([eb, np.zeros((1, 128), F32)], axis=0)
    shared = dict(dw=dw, db=db, ew_ext=ew_ext, ew_t=ew_t, eb_ext=eb_ext,
                  ebt=np.ascontiguousarray(eb.T), sew=sew, sdw=sdw,
                  sebcol=np.ascontiguousarray(seb.reshape(DIM, 1)),
                  sdbcol=np.ascontiguousarray(sdb.reshape(DIM, 1)),
                  tsel=H["typesel"])
    in_maps = []
    for c in range(NC):
        cd = H["cores"][c]
        im = dict(shared)
        im["dv"] = np.ascontiguousarray(
            dv[:, c * Vs:(c + 1) * Vs, :].reshape(2 * Vs, VD))
        im["gidx"] = cd["gidx"]
        im["lmask"] = cd["lmask"]
        im["pmat"] = cd["P"]
        im["hcm"] = cd["hcm"]
        im["adj"] = cd["adj"]
        im["hmat"] = cd["hmat"]
        im["pewidx"] = cd["pewidx"]
        im["ebsel"] = cd["ebsel"]
        im["pmask"] = cd["pmask"]
        im["cmat"] = cd["C"]
        im["hpos"] = cd["hpos"]
        in_maps.append(im)

    if os.environ.get("BASS_KERNEL_SIM") == "1":
        from concourse.bass_interp import MultiCoreSim
        sim = MultiCoreSim(nc, num_cores=NC)
        for c in range(NC):
            for k, v in in_maps[c].items():
                sim.cores[c].tensor(k)[:] = v
        sim.simulate()
        outs = [np.array(sim.cores[c].tensor("out")) for c in range(NC)]
    else:
        from concourse.bass_utils import run_bass_kernel_spmd
        trace = os.environ.get("BASS_KERNEL_TRACE") == "1"
        res = run_bass_kernel_spmd(nc, in_maps, list(range(NC)), trace=trace)
        LAST_EXEC_NS = res.exec_time_ns
        outs = [res.results[c]["out"] for c in range(NC)]

    return np.concatenate([o.reshape(E) for o in outs]).reshape(G * E, 1).astype(F32)
